# revision 2
# baseline (speedup 1.0000x reference)
"""GuidedCrossAttention Trainium2 kernel.

Sharding: 16 graphs -> 8 cores, 2 graphs per core (sorted batch indices make
graphs contiguous row-ranges). Per core we run block-diagonal attention on its
two graphs only. All projections are host-folded into single effective
matrices:
  q2 = xq @ Wq_eff + bq_eff      (SCALE folded in)
  k2 = xk @ Wk_eff + bk_eff
  v2 = xk @ Wv_eff               (v-bias folded into the residual via out-proj)
  out = ctx @ Wout_eff (+ bout folded into the residual term on host)

Device pipeline (feature-major activations so every matmul streams directly):
  q2T/k2T feature-major, v2 token-major with an appended valid-key column.
  S^T[k,q] per (graph, head, key-tile) -> exp (no max-subtract: |scores| << 1
  by construction) -> U = [v2; valid]^T @ E gives unnormalized ctx^T rows and
  the softmax denominator row in one accumulation. Normalization happens as a
  fused copy (U * broadcast(1/d)). Final projection back to token-major, then
  residual + LayerNorm.
"""

import math
from contextlib import ExitStack

import numpy as np

import concourse.bass as bass
import concourse.tile as tile
from concourse import bacc, mybir
from concourse.bass_utils import run_bass_kernel_spmd

QD, KD, HID, NH = 256, 320, 256, 8
NQ, NK, NB = 4096, 4096, 16
DH = HID // NH
EPS = 1e-5
SCALE = 1.0 / math.sqrt(DH)
NCORES = 8
GPC = NB // NCORES  # graphs per core
F32 = mybir.dt.float32
PASS_BARRIERS = True
PHASE_BARRIERS = True
DMA_BIG = "gpsimd"
ATTN_ON = True
NORM_ON = True
OUT_ON = True
PROJ_ON = True


def _ceil(a, b):
    return -(-a // b)


def _nsplits(total, step=512):
    return [(a, min(a + step, total)) for a in range(0, total, step)]


def _build_program(QB, KB, NQC, NQCP, KBC):
    KTC = KBC // 128  # key tiles per core (both graphs)
    KT = KB // 128  # key tiles per graph
    QT = NQCP // 128  # token-major query tiles

    nc = bacc.Bacc(
        "TRN2", target_bir_lowering=False, debug=False, num_devices=NCORES
    )
    xqT_d = nc.declare_dram_parameter("xqT", [QD, NQC], F32, isOutput=False)
    xqtok_d = nc.declare_dram_parameter("xqtok", [NQCP, QD], F32, isOutput=False)
    xkT_d = nc.declare_dram_parameter("xkT", [KD + 1, KBC], F32, isOutput=False)
    wq_d = nc.declare_dram_parameter("wq", [QD, 384], F32, isOutput=False)
    wk_d = nc.declare_dram_parameter("wk", [KD + 1, 384], F32, isOutput=False)
    wv_d = nc.declare_dram_parameter("wv", [KD + 1, NH * (DH + 1)], F32, isOutput=False)
    wo_d = nc.declare_dram_parameter("wo", [HID, QD], F32, isOutput=False)
    bq_d = nc.declare_dram_parameter("bq", [384], F32, isOutput=False)
    bk_d = nc.declare_dram_parameter("bk", [384], F32, isOutput=False)
    lng_d = nc.declare_dram_parameter("lng", [QD], F32, isOutput=False)
    lnb_d = nc.declare_dram_parameter("lnb", [QD], F32, isOutput=False)
    out_d = nc.declare_dram_parameter("out", [NQCP, QD], F32, isOutput=True)

    kchunks = [(0, 128), (128, 256), (256, KD + 1)]
    _barrier_noop = lambda: None  # KD+1=321 partition chunks (valid row)

    with tile.TileContext(nc) as tc, ExitStack() as ctx:
        _dma_big = getattr(nc, DMA_BIG)
        _pass_bar = tc.strict_bb_all_engine_barrier if PASS_BARRIERS else _barrier_noop
        _phase_bar = tc.strict_bb_all_engine_barrier if PHASE_BARRIERS else _barrier_noop
        P = ctx.enter_context(tc.tile_pool(name="persist", bufs=1))

        # ---- constant / activation loads ----
        xqT = [P.tile([128, NQC], F32, tag=f"xqT{i}", name=f"xqT{i}") for i in range(2)]
        for i in range(2):
            _dma_big.dma_start(out=xqT[i], in_=xqT_d[128 * i : 128 * (i + 1), :])
        xkT = []
        for i, (a, b) in enumerate(kchunks):
            t = P.tile([b - a, KBC], F32, tag=f"xkT{i}", name=f"xkT{i}")
            xkT.append(t)
            _dma_big.dma_start(out=t, in_=xkT_d[a:b, :])
        wq = [P.tile([128, 384], F32, tag=f"wq{i}", name=f"wq{i}") for i in range(2)]
        wo = [P.tile([32, QD], F32, tag=f"wo{i}", name=f"wo{i}") for i in range(NH)]
        for i in range(2):
            _dma_big.dma_start(out=wq[i], in_=wq_d[128 * i : 128 * (i + 1), :])
        for i in range(NH):
            _dma_big.dma_start(out=wo[i], in_=wo_d[DH * i : DH * (i + 1), :])
        wk, wv = [], []
        for i, (a, b) in enumerate(kchunks):
            tk = P.tile([b - a, 384], F32, tag=f"wk{i}")
            tv = P.tile([b - a, NH * (DH + 1)], F32, tag=f"wv{i}")
            wk.append(tk)
            wv.append(tv)
            _dma_big.dma_start(out=tk, in_=wk_d[a:b, :])
            _dma_big.dma_start(out=tv, in_=wv_d[a:b, :])
        bq = [P.tile([128, 1], F32, tag=f"bq{i}", name=f"bq{i}") for i in range(3)]
        bk = [P.tile([128, 1], F32, tag=f"bk{i}", name=f"bk{i}") for i in range(3)]
        for i in range(3):
            nc.gpsimd.dma_start(out=bq[i], in_=bq_d[128 * i : 128 * (i + 1)])
            nc.gpsimd.dma_start(out=bk[i], in_=bk_d[128 * i : 128 * (i + 1)])
        lng = P.tile([128, QD], F32, tag="lng")
        lnb = P.tile([128, QD], F32, tag="lnb")
        nc.sync.dma_start(
            out=lng,
            in_=bass.AP(
                tensor=lng_d.ap().tensor, offset=0, ap=[[0, 128], [1, QD]]
            ),
        )
        nc.sync.dma_start(
            out=lnb,
            in_=bass.AP(
                tensor=lnb_d.ap().tensor, offset=0, ap=[[0, 128], [1, QD]]
            ),
        )
        epst = P.tile([128, 1], F32, tag="epst")
        nc.vector.memset(epst, EPS)
        xqtok = [P.tile([128, QD], F32, tag=f"xqtok{i}", name=f"xqtok{i}") for i in range(QT)]
        for i in range(QT):
            _dma_big.dma_start(out=xqtok[i], in_=xqtok_d[128 * i : 128 * (i + 1), :])

        q2T = [P.tile([128, NQC], F32, tag=f"q2T{i}", name=f"q2T{i}") for i in range(3)]
        k2T = [P.tile([128, KBC], F32, tag=f"k2T{i}", name=f"k2T{i}") for i in range(3)]
        v2e = [P.tile([128, NH, DH + 1], F32, tag=f"v2e{i}", name=f"v2e{i}") for i in range(KTC)]
        ctxT = [P.tile([32, NQCP], F32, tag=f"ctxT{i}", name=f"ctxT{i}") for i in range(NH)]
        for i in range(NH):
            nc.vector.memset(ctxT[i], 0.0)

        # ---- projections ----
        with tc.tile_pool(name="proj_ps", bufs=2, space="PSUM") as pp:
            with tc.tile_pool(name="junk_ps", bufs=1, space="PSUM") as jp:
                junk = jp.tile([1, 1], F32, tag="junk", name="junk")
                for t in [xqT[0], xqT[1], *xkT, *wq, *wk, *wv]:
                    nc.tensor.matmul(
                        junk,
                        lhsT=t[0:1, 0:1],
                        rhs=epst[0:1, 0:1],
                        start=True,
                        stop=True,
                        skip_group_check=True,
                    )
            for mc in (range(3) if PROJ_ON else []):
                ps = pp.tile([128, NQC], F32, tag="qk_ps")
                for kc in range(2):
                    for n0, n1 in _nsplits(NQC):
                        nc.tensor.matmul(
                            ps[:, n0:n1],
                            lhsT=wq[kc][:, 128 * mc : 128 * (mc + 1)],
                            rhs=xqT[kc][:, n0:n1],
                            start=(kc == 0),
                            stop=(kc == 1),
                        )
                nc.vector.tensor_scalar(
                    out=q2T[mc],
                    in0=ps,
                    scalar1=bq[mc][:, 0:1],
                    scalar2=None,
                    op0=mybir.AluOpType.add,
                )
            for mc in (range(3) if PROJ_ON else []):
                ps = pp.tile([128, KBC], F32, tag="qk_ps")
                for kc in range(3):
                    for n0, n1 in _nsplits(KBC):
                        nc.tensor.matmul(
                            ps[:, n0:n1],
                            lhsT=wk[kc][:, 128 * mc : 128 * (mc + 1)],
                            rhs=xkT[kc][:, n0:n1],
                            start=(kc == 0),
                            stop=(kc == 2),
                        )
                nc.vector.tensor_scalar(
                    out=k2T[mc],
                    in0=ps,
                    scalar1=bk[mc][:, 0:1],
                    scalar2=None,
                    op0=mybir.AluOpType.add,
                )
            for kt in (range(KTC) if PROJ_ON else []):
                ps = pp.tile([128, NH * (DH + 1)], F32, tag="v_ps")
                for kc in range(3):
                    nc.tensor.matmul(
                        ps,
                        lhsT=xkT[kc][:, 128 * kt : 128 * (kt + 1)],
                        rhs=wv[kc],
                        start=(kc == 0),
                        stop=(kc == 2),
                    )
                nc.vector.tensor_copy(
                    out=v2e[kt].rearrange("p h d -> p (h d)"),
                    in_=ps,
                )

        # ---- attention: per (graph, half-of-heads) pass ----
        with (
            tc.tile_pool(name="s_ps", bufs=2, space="PSUM") as sp,
            tc.tile_pool(name="u_ps", bufs=4, space="PSUM") as up,
            tc.tile_pool(name="e_sb", bufs=3) as ep,
            tc.tile_pool(name="d_sb", bufs=2) as dp,
        ):
            passes = [(g, half) for g in range(GPC) for half in range(2)]
            prev_pass = {}
            for pi, (g, half) in enumerate(passes if ATTN_ON else []):
                    Us = [up.tile([DH + 1, 512], F32, tag="U", name="U") for _ in range(4)]
                    if pi > 0:
                        pg, ph = passes[pi - 1]
                        for j in range(4):
                            hprev = ph * 4 + j
                            nc.tensor.matmul(
                                Us[j][0:1, 0:1],
                                lhsT=ctxT[hprev][0:1, pg * QB : pg * QB + 1],
                                rhs=epst[0:1, 0:1],
                                start=True,
                                stop=True,
                                skip_group_check=True,
                            )
                    for hp in range(2):
                        for kt in range(KT):
                            S = sp.tile([128, 2, 512], F32, tag="S")
                            for j2 in range(2):
                                h = half * 4 + hp * 2 + j2
                                mc, r = h // 3, (h % 3) * DH
                                nc.tensor.matmul(
                                    S[:, j2, 0:QB],
                                    lhsT=k2T[mc][
                                        r : r + DH,
                                        g * KB + 128 * kt : g * KB + 128 * (kt + 1),
                                    ],
                                    rhs=q2T[mc][r : r + DH, g * QB : (g + 1) * QB],
                                    start=True,
                                    stop=True,
                                )
                            E = ep.tile([128, 2, QB], F32, tag="E")
                            nc.scalar.activation(
                                out=E,
                                in_=S[:, :, 0:QB],
                                func=mybir.ActivationFunctionType.Exp,
                            )
                            for j2 in range(2):
                                j = hp * 2 + j2
                                nc.tensor.matmul(
                                    Us[j][:, 0:QB],
                                    lhsT=v2e[g * KT + kt][:, half * 4 + j, :],
                                    rhs=E[:, j2, :],
                                    start=(kt == 0),
                                    stop=(kt == KT - 1),
                                )
                    for j in (range(4) if NORM_ON else []):
                        h = half * 4 + j
                        rdt = dp.tile([DH + 1, QB], F32, tag=f"rdt{j}", name=f"rdt{j}")
                        nc.vector.reciprocal(
                            out=rdt[DH : DH + 1, :], in_=Us[j][DH : DH + 1, 0:QB]
                        )
                        d0 = dp.tile([1, QB], F32, tag=f"d0{j}", name=f"d0{j}")
                        nc.sync.dma_start(out=d0, in_=rdt[DH : DH + 1, :])
                        rep = dp.tile([DH, QB], F32, tag=f"rep{j}", name=f"rep{j}")
                        nc.gpsimd.partition_broadcast(rep, d0[0:1, :], channels=DH)
                        nc.vector.scalar_tensor_tensor(
                            out=ctxT[h][:, g * QB : (g + 1) * QB],
                            in0=Us[j][0:DH, 0:QB],
                            scalar=0.0,
                            in1=rep,
                            op0=mybir.AluOpType.bypass,
                            op1=mybir.AluOpType.mult,
                        )


        # ---- out-projection + residual + layernorm ----
        with (
            tc.tile_pool(name="o_ps", bufs=2, space="PSUM") as op,
            tc.tile_pool(name="ln_sb", bufs=3) as lp,
        ):
            for qt in (range(QT) if OUT_ON else []):
                ps = op.tile([128, QD], F32, tag="o_ps")
                for h in range(NH):
                    if h == 0:
                        nc.tensor.matmul(
                            ps[0:1, 0:1],
                            lhsT=ctxT[7][0:1, 128 * qt : 128 * qt + 1],
                            rhs=epst[0:1, 0:1],
                            start=True,
                            stop=True,
                            skip_group_check=True,
                        )
                    nc.tensor.matmul(
                        ps,
                        lhsT=ctxT[h][:, 128 * qt : 128 * (qt + 1)],
                        rhs=wo[h],
                        start=(h == 0),
                        stop=(h == NH - 1),
                    )
                x = lp.tile([128, QD], F32, tag="x")
                nc.vector.tensor_add(x, ps, xqtok[qt])
                stats = lp.tile([128, 6], F32, tag="stats")
                nc.vector.bn_stats(out=stats, in_=x)
                mv = lp.tile([128, 2], F32, tag="mv")
                nc.vector.bn_aggr(out=mv, in_=stats)
                sd = lp.tile([128, 1], F32, tag="sd")
                nc.scalar.activation(
                    out=sd, in_=mv[:, 1:2], func=mybir.ActivationFunctionType.Sqrt,
                    bias=epst[:, 0:1],
                )
                rstd = lp.tile([128, 1], F32, tag="rstd")
                nc.vector.reciprocal(out=rstd, in_=sd)
                xc = lp.tile([128, QD], F32, tag="xc")
                nc.vector.tensor_scalar(
                    out=xc,
                    in0=x,
                    scalar1=mv[:, 0:1],
                    scalar2=None,
                    op0=mybir.AluOpType.subtract,
                )
                y = lp.tile([128, QD], F32, tag="y")
                nc.vector.scalar_tensor_tensor(
                    out=y,
                    in0=xc,
                    scalar=rstd[:, 0:1],
                    in1=lng,
                    op0=mybir.AluOpType.mult,
                    op1=mybir.AluOpType.mult,
                )
                yb = lp.tile([128, QD], F32, tag="yb")
                nc.gpsimd.tensor_add(yb, y, lnb)
                nc.sync.dma_start(out=out_d[128 * qt : 128 * (qt + 1), :], in_=yb)

    nc.compile()
    return nc


def kernel(**inputs):
    xq = np.ascontiguousarray(np.asarray(inputs["query_nodes"], dtype=np.float32))
    xk = np.ascontiguousarray(np.asarray(inputs["key_nodes"], dtype=np.float32))
    qbi = np.asarray(inputs["query_batch_idx"]).astype(np.int64)
    kbi = np.asarray(inputs["key_batch_idx"]).astype(np.int64)
    Wq = np.asarray(inputs["Wq"], np.float32)
    Wk = np.asarray(inputs["Wk"], np.float32)
    Wv = np.asarray(inputs["Wv"], np.float32)
    bq0 = np.asarray(inputs["bq"], np.float32)
    bk0 = np.asarray(inputs["bk"], np.float32)
    bv0 = np.asarray(inputs["bv"], np.float32)
    W2 = np.asarray(inputs["in_proj_w"], np.float32)
    b2 = np.asarray(inputs["in_proj_b"], np.float32)
    mow = np.asarray(inputs["mha_ow"], np.float32)
    mob = np.asarray(inputs["mha_ob"], np.float32)
    Wo = np.asarray(inputs["Wo"], np.float32)
    bo = np.asarray(inputs["bo"], np.float32)
    lng = np.asarray(inputs["ln_g"], np.float32)
    lnb = np.asarray(inputs["ln_b"], np.float32)

    # host-side weight folding
    Wq_eff = (Wq @ W2[:HID].T) * SCALE
    bq_eff = (bq0 @ W2[:HID].T + b2[:HID]) * SCALE
    Wk_eff = Wk @ W2[HID : 2 * HID].T
    bk_eff = bk0 @ W2[HID : 2 * HID].T + b2[HID : 2 * HID]
    Wv_eff = Wv @ W2[2 * HID :].T
    bv_eff = bv0 @ W2[2 * HID :].T + b2[2 * HID :]
    Wout_eff = mow @ Wo
    bout = bv_eff @ Wout_eff + mob @ Wo + bo  # folded into residual

    qcnt = np.bincount(qbi, minlength=NB)
    kcnt = np.bincount(kbi, minlength=NB)
    qoff = np.concatenate([[0], np.cumsum(qcnt)])
    koff = np.concatenate([[0], np.cumsum(kcnt)])

    QB = int(_ceil(max(int(qcnt.max()), 8), 8) * 8)
    KB = int(_ceil(max(int(kcnt.max()), 1), 128) * 128)
    NQC = GPC * QB
    NQCP = _ceil(NQC, 128) * 128
    KBC = GPC * KB

    nc = _build_program(QB, KB, NQC, NQCP, KBC)

    # pack 8 heads as 3-per-128-partition-tile (PE base-partition must be 0/32/64)
    def _headpack_cols(W):
        Wp = np.zeros((W.shape[0], 384), np.float32)
        for h in range(NH):
            Wp[:, 128 * (h // 3) + DH * (h % 3) : 128 * (h // 3) + DH * (h % 3) + DH] = (
                W[:, DH * h : DH * (h + 1)]
            )
        return Wp

    def _headpack_vec(v):
        vp = np.zeros((384,), np.float32)
        for h in range(NH):
            vp[128 * (h // 3) + DH * (h % 3) : 128 * (h // 3) + DH * (h % 3) + DH] = v[
                DH * h : DH * (h + 1)
            ]
        return vp

    wqT = _headpack_cols(Wq_eff)
    wkT = np.zeros((KD + 1, 384), np.float32)
    wkT[:KD] = _headpack_cols(Wk_eff)
    wvT = np.zeros((KD + 1, NH * (DH + 1)), np.float32)
    for h in range(NH):
        wvT[:KD, (DH + 1) * h : (DH + 1) * h + DH] = Wv_eff[:, DH * h : DH * (h + 1)]
        wvT[KD, (DH + 1) * h + DH] = 1.0
    woT = np.ascontiguousarray(Wout_eff)
    bq_eff = _headpack_vec(bq_eff)
    bk_eff = _headpack_vec(bk_eff)

    in_maps = []
    for c in range(NCORES):
        xqT = np.zeros((QD, NQC), np.float32)
        xqtok = np.zeros((NQCP, QD), np.float32)
        xkT = np.zeros((KD + 1, KBC), np.float32)
        for gi in range(GPC):
            g = GPC * c + gi
            nq = int(qcnt[g])
            nk = int(kcnt[g])
            if nq:
                rows = xq[qoff[g] : qoff[g + 1]]
                xqT[:, gi * QB : gi * QB + nq] = rows.T
                xqtok[gi * QB : gi * QB + nq] = rows + bout
            if nk:
                xkT[:KD, gi * KB : gi * KB + nk] = xk[koff[g] : koff[g + 1]].T
                xkT[KD, gi * KB : gi * KB + nk] = 1.0
        in_maps.append(
            {
                "xqT": xqT,
                "xqtok": xqtok,
                "xkT": xkT,
                "wq": wqT,
                "wk": wkT,
                "wv": wvT,
                "wo": woT,
                "bq": bq_eff.copy(),
                "bk": bk_eff.copy(),
                "lng": lng.copy(),
                "lnb": lnb.copy(),
            }
        )

    import os

    trace = bool(os.environ.get("BASS_TRACE"))
    tmpdir = os.environ.get("BASS_TRACE_DIR") or None
    if tmpdir:
        import shutil

        shutil.rmtree(tmpdir, ignore_errors=True)
        os.makedirs(tmpdir, exist_ok=True)
    res = run_bass_kernel_spmd(
        nc, in_maps, list(range(NCORES)), trace=trace, tmpdir=tmpdir
    )
    if getattr(res, "exec_time_ns", None):
        print(f"HW exec time: {res.exec_time_ns} ns")
    out = np.empty((NQ, QD), np.float32)
    for c in range(NCORES):
        oc = res.results[c]["out"]
        for gi in range(GPC):
            g = GPC * c + gi
            nq = int(qcnt[g])
            if nq:
                out[qoff[g] : qoff[g + 1]] = oc[gi * QB : gi * QB + nq]
    return out

